# revision 16
# baseline (speedup 1.0000x reference)
"""Trainium2 Bass kernel for a 2-layer GRU decoder (B=128, T=512, H=512).

Sharding: data-parallel on batch across 8 cores (16 batch elems/core).

Architecture (v2): both GRU layers run interleaved on every core, with
layer 6 lagging layer 5 by one 32-step chunk, so the two independent
per-step dependency chains (matmul -> gate math -> h') fill each
other's engine bubbles.  All intermediates (xW5, g5, xW6, g6) live in
SBUF rings -- no DRAM roundtrips.  xW bulk matmuls + dense output are
computed per-chunk inside the same loop.  The phase-1 contraction is
reduced 320->80 via  mask*concat(z,x2) @ W5
                    = (mask*x2) @ W5[x2-rows] + (mask*onehot) @ (z@W5[z-rows]).

Gate math per step (per layer) is slimmed to 6 DVE ops + 1 ACT (tanh)
+ 2 GPSIMD ops, with z|r merged into a single PSUM tile so one add and
one clip cover both gates.  h is carried in bf16 (the z-gate convex
blend damps rounding noise).
"""

import numpy as np
import ml_dtypes

B, T, LAT, F2, H = 128, 512, 256, 64, 512
G3 = 3 * H              # 1536
NCORES = 8
BL = B // NCORES        # 16 batch / core
NT = T * BL             # 8192 flat (t,b) cols per core
KH = H // 128           # 4 hidden-dim chunks
NCH = G3 // 128         # 12 gate-dim chunks (0-3 z, 4-7 r, 8-11 ih)
C = 32                  # timesteps per chunk
NCHK = T // C           # 16 chunks

bf16 = ml_dtypes.bfloat16
_CACHE = {}


def _build(bd_val, has_brh):
    import concourse.bass as bass
    import concourse.tile as tile
    import concourse.mybir as mybir
    from concourse import bacc
    from concourse.bass import ds

    f32 = mybir.dt.float32
    bf = mybir.dt.bfloat16
    AF = mybir.ActivationFunctionType
    OP = mybir.AluOpType
    ET = mybir.EngineType

    nc = bacc.Bacc(None, target_bir_lowering=False, debug=False)

    xt_d = nc.dram_tensor("xt_d", [128, NT], bf, kind="ExternalInput")
    w5a_d = nc.dram_tensor("w5a_d", [128, G3], bf, kind="ExternalInput")
    u5_d = nc.dram_tensor("u5_d", [KH, 128, G3], bf, kind="ExternalInput")
    w6_d = nc.dram_tensor("w6_d", [KH, 128, G3], bf, kind="ExternalInput")
    u6_d = nc.dram_tensor("u6_d", [KH, 128, G3], bf, kind="ExternalInput")
    b5_d = nc.dram_tensor("b5_d", [128, NCH], f32, kind="ExternalInput")
    b6_d = nc.dram_tensor("b6_d", [128, NCH], f32, kind="ExternalInput")
    br5_d = nc.dram_tensor("br5_d", [128, KH], f32, kind="ExternalInput")
    br6_d = nc.dram_tensor("br6_d", [128, KH], f32, kind="ExternalInput")
    wd_d = nc.dram_tensor("wd_d", [128, KH], bf, kind="ExternalInput")
    dm_d = nc.dram_tensor("dm_d", [1, NT], f32, kind="ExternalInput")
    out_d = nc.dram_tensor("out_d", [1, NT], f32, kind="ExternalOutput")

    with tile.TileContext(nc) as tc:
        import contextlib
        stack = contextlib.ExitStack()
        with stack:
            pp = stack.enter_context(tc.tile_pool(name="persist", bufs=1))
            u5_sb = pp.tile([128, KH, G3], bf, tag="u5")
            u6_sb = pp.tile([128, KH, G3], bf, tag="u6")
            w6_sb = pp.tile([128, KH, G3], bf, tag="w6")
            w5a_sb = pp.tile([128, G3], bf, tag="w5a")
            b5_sb = pp.tile([128, NCH], f32, tag="b5")
            b6_sb = pp.tile([128, NCH], f32, tag="b6")
            wd_sb = pp.tile([128, KH], bf, tag="wd")
            hz = pp.tile([128, KH, BL], bf, tag="hz")
            # double-buffered rings (indexed by chunk parity)
            xa_sb = [pp.tile([128, 512], bf, tag=f"xa{r}", name=f"xa{r}")
                     for r in range(2)]
            xw5_sb = [pp.tile([128, C, NCH, BL], bf, tag=f"xw5{r}", name=f"xw5{r}")
                      for r in range(2)]
            xw6_sb = [pp.tile([128, C, NCH, BL], bf, tag=f"xw6{r}", name=f"xw6{r}")
                      for r in range(2)]
            h5_sb = [pp.tile([128, KH, C, BL], bf, tag=f"h5{r}", name=f"h5{r}")
                     for r in range(2)]
            h6_sb = [pp.tile([128, KH, C, BL], bf, tag=f"h6{r}", name=f"h6{r}")
                     for r in range(2)]
            dmc_sb = [pp.tile([1, 512], f32, tag=f"dm{r}", name=f"dmc{r}")
                      for r in range(2)]
            dec_sb = [pp.tile([1, 512], f32, tag=f"dec{r}", name=f"dec{r}")
                      for r in range(2)]

            nc.sync.dma_start(u5_sb[:], u5_d.ap().rearrange("k p g -> p k g"))
            nc.sync.dma_start(u6_sb[:], u6_d.ap().rearrange("k p g -> p k g"))
            nc.sync.dma_start(w6_sb[:], w6_d.ap().rearrange("k p g -> p k g"))
            nc.sync.dma_start(w5a_sb[:], w5a_d.ap()[:])
            nc.sync.dma_start(b5_sb[:], b5_d.ap()[:])
            nc.sync.dma_start(b6_sb[:], b6_d.ap()[:])
            nc.sync.dma_start(wd_sb[:], wd_d.ap()[:])
            nc.vector.memset(hz[:], 0.0)
            if has_brh:
                brh5_sb = pp.tile([128, KH], f32, tag="brh5")
                brh6_sb = pp.tile([128, KH], f32, tag="brh6")
                nc.sync.dma_start(brh5_sb[:], br5_d.ap()[:])
                nc.sync.dma_start(brh6_sb[:], br6_d.ap()[:])

            psb = stack.enter_context(
                tc.tile_pool(name="psbulk", bufs=2, space="PSUM"))
            psd = stack.enter_context(
                tc.tile_pool(name="psdense", bufs=1, space="PSUM"))
            prec5 = stack.enter_context(
                tc.tile_pool(name="prec5", bufs=2, space="PSUM"))
            prec6 = stack.enter_context(
                tc.tile_pool(name="prec6", bufs=2, space="PSUM"))
            wk = stack.enter_context(tc.tile_pool(name="wk", bufs=3))

            def bulk5(ring, colbase):
                """xW5 chunk: xaug @ W5aug + b5 -> xw5_sb[ring] (K=1 chunk)."""
                nc.sync.dma_start(
                    xa_sb[ring][:], xt_d.ap()[:, ds(colbase, 512)])
                for cc in range(NCH):
                    ps = psb.tile([128, 512], f32, tag="psb")
                    nc.tensor.matmul(
                        ps[:], w5a_sb[:, cc * 128:(cc + 1) * 128],
                        xa_sb[ring][:], start=True, stop=True,
                    )
                    nc.scalar.activation(
                        xw5_sb[ring][:, :, cc, :],
                        ps[:].rearrange("p (t b) -> p t b", b=BL),
                        AF.Identity, bias=b5_sb[:, cc:cc + 1],
                    )

            def bulk6(ring):
                """xW6 chunk: g5(chunk) @ W6 + b6 -> xw6_sb[ring]."""
                for cc in range(NCH):
                    ps = psb.tile([128, 512], f32, tag="psb")
                    for k in range(KH):
                        nc.tensor.matmul(
                            ps[:], w6_sb[:, k, cc * 128:(cc + 1) * 128],
                            h5_sb[ring][:, k, :, :]
                            .rearrange("p t b -> p (t b)"),
                            start=(k == 0), stop=(k == KH - 1),
                        )
                    nc.scalar.activation(
                        xw6_sb[ring][:, :, cc, :],
                        ps[:].rearrange("p (t b) -> p t b", b=BL),
                        AF.Identity, bias=b6_sb[:, cc:cc + 1],
                    )

            def dense(ring, colbase):
                """dec chunk: tanh(g6 @ Wd + bd) * dec_mask -> out_d."""
                nc.sync.dma_start(
                    dmc_sb[ring][:], dm_d.ap()[:, ds(colbase, 512)])
                ps = psd.tile([1, 512], f32, tag="psd")
                for k in range(KH):
                    nc.tensor.matmul(
                        ps[:], wd_sb[:, k:k + 1],
                        h6_sb[ring][:, k, :, :].rearrange("p t b -> p (t b)"),
                        start=(k == 0), stop=(k == KH - 1),
                    )
                nc.scalar.activation(
                    dec_sb[ring][:], ps[:], AF.Tanh, bias=float(bd_val))
                nc.gpsimd.tensor_tensor(
                    dec_sb[ring][:], dec_sb[ring][:], dmc_sb[ring][:],
                    op=OP.mult)
                nc.sync.dma_start(
                    out_d.ap()[:, ds(colbase, 512)], dec_sb[ring][:])

            def mm_step(lay, ring, uu, hprev):
                """Emit the 48 recurrent matmuls of one step (PE only)."""
                u_sb = u5_sb if lay == 5 else u6_sb
                prec = prec5 if lay == 5 else prec6
                ps_rec = prec.tile([128, 3, KH, BL], f32, tag="psrec")
                ps_zr = ps_rec[:, 0:2, :, :]
                ps_ih = ps_rec[:, 2, :, :]
                first = True
                for g in range(2):          # z, r
                    for cg in range(KH):
                        for k in range(KH):
                            nc.tensor.matmul(
                                ps_zr[:, g, cg, :],
                                u_sb[:, k, g * 512 + cg * 128:
                                     g * 512 + (cg + 1) * 128],
                                hprev(k),
                                start=first,
                                stop=(g == 1 and cg == KH - 1
                                      and k == KH - 1),
                                skip_group_check=True,
                            )
                            first = False
                for cg in range(KH):
                    for k in range(KH):
                        nc.tensor.matmul(
                            ps_ih[:, cg, :],
                            u_sb[:, k, 1024 + cg * 128:1024 + (cg + 1) * 128],
                            hprev(k),
                            start=(cg == 0 and k == 0),
                            stop=(cg == KH - 1 and k == KH - 1),
                            skip_group_check=True,
                        )
                return ps_zr, ps_ih

            def gates_a(lay, ring, uu, ps_zr):
                """z|r merged add + clip -> gzr bf16 (cols: z then r).

                The add reads PSUM so it must run on DVE; layer 6's clip
                runs on gpsimd so L6's chain never head-of-line-blocks
                L5's post-matmul chain in the DVE queue."""
                xw = xw5_sb[ring] if lay == 5 else xw6_sb[ring]
                eng = nc.vector if lay == 5 else nc.gpsimd
                szr = wk.tile([128, 2, KH, BL], bf, tag=f"szr{lay}")
                nc.vector.tensor_tensor(
                    szr[:],
                    xw[:, uu, 0:8, :].rearrange("p (g k) b -> p g k b", g=2),
                    ps_zr[:],
                    op=OP.add,
                )
                gzr = wk.tile([128, 2, KH, BL], bf, tag=f"gzr{lay}")
                eng.tensor_scalar(
                    gzr[:], szr[:], 0.0, 1.0, op0=OP.max, op1=OP.min,
                )
                return gzr

            def gates_b(lay, ring, uu, ps_ih, gzr, hprev_full, brh_sb):
                """q, hpre -> tanh; ug/vg on gpsimd."""
                xw = xw5_sb[ring] if lay == 5 else xw6_sb[ring]
                eng = nc.vector if lay == 5 else nc.gpsimd
                sh = nc.gpsimd if lay == 5 else nc.vector
                q = wk.tile([128, KH, BL], bf, tag=f"q{lay}")
                if brh_sb is not None:
                    nc.vector.scalar_tensor_tensor(
                        q[:], ps_ih[:], 1.0,
                        brh_sb[:].rearrange("p k -> p k 1").broadcast(2, BL),
                        op0=OP.mult, op1=OP.add,
                    )
                    nc.vector.tensor_tensor(
                        q[:], gzr[:, 1, :, :], q[:], op=OP.mult)
                else:
                    nc.vector.tensor_tensor(
                        q[:], gzr[:, 1, :, :], ps_ih[:], op=OP.mult)
                hpre = wk.tile([128, KH, BL], bf, tag=f"hpre{lay}")
                eng.tensor_tensor(
                    hpre[:], q[:], xw[:, uu, 8:12, :], op=OP.add)
                hh = wk.tile([128, KH, BL], bf, tag=f"hh{lay}")
                nc.scalar.activation(hh[:], hpre[:], AF.Tanh)
                # off-spine blend prep, runs in tanh/q shadow on the
                # opposite engine of this layer's chain
                vg = wk.tile([128, KH, BL], bf, tag=f"vg{lay}")
                sh.tensor_scalar(
                    vg[:], gzr[:, 0, :, :], -1.0, 1.0,
                    op0=OP.mult, op1=OP.add,
                )
                ug = wk.tile([128, KH, BL], bf, tag=f"ug{lay}")
                sh.tensor_tensor(
                    ug[:], gzr[:, 0, :, :], hprev_full, op=OP.mult)
                return hh, ug, vg

            def gates_c(lay, ring, uu, hh, ug, vg):
                """ee = vg*hh ; h' = ug + ee -> hist[uu] (bf16)."""
                hist = h5_sb[ring] if lay == 5 else h6_sb[ring]
                eng = nc.vector if lay == 5 else nc.gpsimd
                ee = wk.tile([128, KH, BL], bf, tag=f"ee{lay}")
                eng.tensor_tensor(ee[:], vg[:], hh[:], op=OP.mult)
                eng.tensor_tensor(
                    hist[:, :, uu, :], ug[:], ee[:], op=OP.add)

            def hprev_of(lay, ring, uu, zero):
                hist = h5_sb[ring] if lay == 5 else h6_sb[ring]
                prev = h5_sb[1 - ring] if lay == 5 else h6_sb[1 - ring]
                if uu > 0:
                    return (lambda k: hist[:, k, uu - 1, :],
                            hist[:, :, uu - 1, :])
                if zero:
                    return (lambda k: hz[:, k, :], hz[:])
                return (lambda k: prev[:, k, C - 1, :],
                        prev[:, :, C - 1, :])

            brh5 = brh5_sb if has_brh else None
            brh6 = brh6_sb if has_brh else None

            def rec_pair(r5, r6, z5=False, z6=False, do5=True, do6=True):
                """Interleaved chunk: L5 chunk (ring r5) + L6 chunk (r6)."""
                for uu in range(C):
                    if do5:
                        hp5, hp5f = hprev_of(5, r5, uu, z5)
                        zr5, ih5 = mm_step(5, r5, uu, hp5)
                    if do6:
                        hp6, hp6f = hprev_of(6, r6, uu, z6)
                    if do5:
                        g5 = gates_a(5, r5, uu, zr5)
                    if do6:
                        zr6, ih6 = mm_step(6, r6, uu, hp6)
                    if do5:
                        hh5, ug5, vg5 = gates_b(5, r5, uu, ih5, g5, hp5f,
                                                brh5)
                        gates_c(5, r5, uu, hh5, ug5, vg5)
                    if do6:
                        g6 = gates_a(6, r6, uu, zr6)
                        hh6, ug6, vg6 = gates_b(6, r6, uu, ih6, g6, hp6f,
                                                brh6)
                        gates_c(6, r6, uu, hh6, ug6, vg6)

            # ---------------- prologue ----------------
            bulk5(0, 0)
            bulk5(1, 512)
            rec_pair(0, 0, z5=True, do6=False)          # L5 chunk 0
            bulk6(0)
            bulk5(0, 1024)                              # chunk 2
            rec_pair(1, 0, z6=True)                     # L5 c1 | L6 c0
            bulk6(1)
            bulk5(1, 3 * 512)                           # chunk 3
            dense(0, 0)                                 # dec chunk 0

            # ---------------- steady loop: i covers L5 chunks 2..13 -------
            with tc.For_i(
                2 * 512, 14 * 512, 2 * 512,
                hint_engines=(ET.PE, ET.DVE, ET.Activation, ET.Pool),
            ) as iv:
                # first half: L5 chunk c (even, ring0) | L6 chunk c-1 (ring1)
                rec_pair(0, 1)
                bulk6(0)
                bulk5(0, iv + 2 * 512)
                dense(1, iv - 512)
                # second half: L5 chunk c+1 (ring1) | L6 chunk c (ring0)
                rec_pair(1, 0)
                bulk6(1)
                bulk5(1, iv + 3 * 512)
                dense(0, iv)

            # ---------------- epilogue ----------------
            rec_pair(0, 1)                              # L5 c14 | L6 c13
            bulk6(0)
            dense(1, 13 * 512)
            rec_pair(1, 0)                              # L5 c15 | L6 c14
            bulk6(1)
            dense(0, 14 * 512)
            rec_pair(1, 1, do5=False)                   # L6 c15
            dense(1, 15 * 512)

    nc.compile()
    return nc


def _prep(inputs):
    """Host-side: shard on batch, permute/pad/cast into device layouts."""
    z = np.asarray(inputs["z"], np.float32)
    x2 = np.asarray(inputs["train_input_two"], np.float32)
    masks = np.asarray(inputs["masks"], np.float32)
    dmasks = np.asarray(inputs["dec_masks"], np.float32)
    W5 = np.asarray(inputs["W5"], np.float32)
    U5 = np.asarray(inputs["U5"], np.float32)
    bi5 = np.asarray(inputs["bi5"], np.float32)
    br5 = np.asarray(inputs["br5"], np.float32)
    W6 = np.asarray(inputs["W6"], np.float32)
    U6 = np.asarray(inputs["U6"], np.float32)
    bi6 = np.asarray(inputs["bi6"], np.float32)
    br6 = np.asarray(inputs["br6"], np.float32)
    Wd = np.asarray(inputs["Wd"], np.float32)
    bd = np.asarray(inputs["bd"], np.float32)

    def scale_w(W):  # scale z,r columns by 0.2 (hard-sigmoid prescale)
        Ws = W.copy()
        Ws[:, : 2 * H] *= 0.2
        return Ws

    def pack_w(W, kdim):  # [D,G3] -> [kdim,128,G3] bf16 (zero-padded)
        D = W.shape[0]
        Wp = np.zeros((kdim * 128, G3), np.float32)
        Wp[:D] = W
        return np.ascontiguousarray(Wp.reshape(kdim, 128, G3).astype(bf16))

    def pack_bias(bi, br):  # xw-path bias, [128, NCH] (partition, chunk)
        bt = np.empty(G3, np.float32)
        bt[: 2 * H] = 0.2 * (bi[: 2 * H] + br[: 2 * H]) + 0.5
        bt[2 * H:] = bi[2 * H:]
        return np.ascontiguousarray(bt.reshape(NCH, 128).T)

    W5s = scale_w(W5)
    u5p = pack_w(scale_w(U5), KH)
    w6p = pack_w(scale_w(W6), KH)
    u6p = pack_w(scale_w(U6), KH)
    b5p = pack_bias(bi5, br5)
    b6p = pack_bias(bi6, br6)
    brh5 = np.ascontiguousarray(br5[2 * H:].reshape(KH, 128).T)
    brh6 = np.ascontiguousarray(br6[2 * H:].reshape(KH, 128).T)
    has_brh = bool(np.any(brh5) or np.any(brh6))
    wdp = np.ascontiguousarray(Wd[:, 0].reshape(KH, 128).T.astype(bf16))

    in_maps = []
    for cidx in range(NCORES):
        sl = slice(cidx * BL, (cidx + 1) * BL)
        # augmented input: rows 0:64 = (mask*x2)^T, 64:80 = mask*onehot(b)
        xm = x2[sl] * masks[sl]                       # [BL,T,64]
        XT = np.zeros((128, T, BL), np.float32)
        XT[:F2] = xm.transpose(2, 1, 0)
        mk = masks[sl, :, 0]                          # [BL,T]
        for b in range(BL):
            XT[F2 + b, :, b] = mk[b]
        # augmented W5: rows 0:64 = W5s[x2 rows], 64:80 = z @ W5s[z rows]
        W5a = np.zeros((128, G3), np.float32)
        W5a[:F2] = W5s[LAT:]
        W5a[F2:F2 + BL] = z[sl] @ W5s[:LAT]
        dmc = dmasks[sl, :, 0].T.reshape(NT)          # flat t*BL+b
        in_maps.append({
            "xt_d": np.ascontiguousarray(
                XT.reshape(128, NT).astype(bf16)),
            "w5a_d": np.ascontiguousarray(W5a.astype(bf16)),
            "u5_d": u5p, "w6_d": w6p, "u6_d": u6p,
            "b5_d": b5p, "b6_d": b6p,
            "br5_d": brh5, "br6_d": brh6,
            "wd_d": wdp,
            "dm_d": np.ascontiguousarray(dmc.reshape(1, NT)),
        })
    return in_maps, has_brh, float(bd.reshape(-1)[0])


def kernel(**inputs):
    from concourse.bass_utils import run_bass_kernel_spmd

    in_maps, has_brh, bd_val = _prep(inputs)
    key = (has_brh, bd_val)
    if key not in _CACHE:
        _CACHE[key] = _build(bd_val, has_brh)
    nc = _CACHE[key]
    res = run_bass_kernel_spmd(nc, in_maps, core_ids=list(range(NCORES)))
    out = np.empty((B, T, 1), np.float32)
    for cidx in range(NCORES):
        flat = res.results[cidx]["out_d"].reshape(NT)  # flat = t*BL + b
        out[cidx * BL:(cidx + 1) * BL, :, 0] = flat.reshape(T, BL).T
    return out


# revision 21
# speedup vs baseline: 1.2988x; 1.2988x over previous
"""Trainium2 Bass kernel for a 2-layer GRU decoder (B=128, T=512, H=512).

Sharding: data-parallel on batch across 8 cores (16 batch elems/core).

Architecture (v2): both GRU layers run interleaved on every core, with
layer 6 lagging layer 5 by one 32-step chunk, so the two independent
per-step dependency chains (matmul -> gate math -> h') fill each
other's engine bubbles.  All intermediates (xW5, g5, xW6, g6) live in
SBUF rings -- no DRAM roundtrips.  xW bulk matmuls + dense output are
computed per-chunk inside the same loop.  The phase-1 contraction is
reduced 320->80 via  mask*concat(z,x2) @ W5
                    = (mask*x2) @ W5[x2-rows] + (mask*onehot) @ (z@W5[z-rows]).

Gate math per step (per layer) is slimmed to 6 DVE ops + 1 ACT (tanh)
+ 2 GPSIMD ops, with z|r merged into a single PSUM tile so one add and
one clip cover both gates.  h is carried in bf16 (the z-gate convex
blend damps rounding noise).
"""

import numpy as np
import ml_dtypes

B, T, LAT, F2, H = 128, 512, 256, 64, 512
G3 = 3 * H              # 1536
NCORES = 8
BL = B // NCORES        # 16 batch / core
NT = T * BL             # 8192 flat (t,b) cols per core
KH = H // 128           # 4 hidden-dim chunks
NCH = G3 // 128         # 12 gate-dim chunks (0-3 z, 4-7 r, 8-11 ih)
C = 32                  # timesteps per chunk
NCHK = T // C           # 16 chunks

bf16 = ml_dtypes.bfloat16
_CACHE = {}


def _build(bd_val, has_brh):
    import concourse.bass as bass
    import concourse.tile as tile
    import concourse.mybir as mybir
    from concourse import bacc
    from concourse.bass import ds

    f32 = mybir.dt.float32
    bf = mybir.dt.bfloat16
    AF = mybir.ActivationFunctionType
    OP = mybir.AluOpType
    ET = mybir.EngineType

    nc = bacc.Bacc(None, target_bir_lowering=False, debug=False)

    xt_d = nc.dram_tensor("xt_d", [128, NT], bf, kind="ExternalInput")
    w5a_d = nc.dram_tensor("w5a_d", [128, G3], bf, kind="ExternalInput")
    u5_d = nc.dram_tensor("u5_d", [KH, 128, G3], bf, kind="ExternalInput")
    w6_d = nc.dram_tensor("w6_d", [KH, 128, G3], bf, kind="ExternalInput")
    u6_d = nc.dram_tensor("u6_d", [KH, 128, G3], bf, kind="ExternalInput")
    b5_d = nc.dram_tensor("b5_d", [128, NCH], f32, kind="ExternalInput")
    b6_d = nc.dram_tensor("b6_d", [128, NCH], f32, kind="ExternalInput")
    br5_d = nc.dram_tensor("br5_d", [128, KH], f32, kind="ExternalInput")
    br6_d = nc.dram_tensor("br6_d", [128, KH], f32, kind="ExternalInput")
    wd_d = nc.dram_tensor("wd_d", [128, KH], bf, kind="ExternalInput")
    dm_d = nc.dram_tensor("dm_d", [1, NT], f32, kind="ExternalInput")
    out_d = nc.dram_tensor("out_d", [1, NT], f32, kind="ExternalOutput")

    with tile.TileContext(nc) as tc:
        import contextlib
        stack = contextlib.ExitStack()
        with stack:
            pp = stack.enter_context(tc.tile_pool(name="persist", bufs=1))
            u5_sb = pp.tile([128, KH, G3], bf, tag="u5")
            u6_sb = pp.tile([128, KH, G3], bf, tag="u6")
            w6_sb = pp.tile([128, KH, G3], bf, tag="w6")
            w5a_sb = pp.tile([128, G3], bf, tag="w5a")
            b5_sb = pp.tile([128, NCH], f32, tag="b5")
            b6_sb = pp.tile([128, NCH], f32, tag="b6")
            wd_sb = pp.tile([128, KH], bf, tag="wd")
            hz = pp.tile([128, KH, BL], bf, tag="hz")
            # double-buffered rings (indexed by chunk parity)
            xa_sb = [pp.tile([128, 512], bf, tag=f"xa{r}", name=f"xa{r}")
                     for r in range(2)]
            xw5_sb = [pp.tile([128, C, NCH, BL], bf, tag=f"xw5{r}", name=f"xw5{r}")
                      for r in range(2)]
            xw6_sb = [pp.tile([128, C, NCH, BL], bf, tag=f"xw6{r}", name=f"xw6{r}")
                      for r in range(2)]
            h5_sb = [pp.tile([128, KH, C, BL], bf, tag=f"h5{r}", name=f"h5{r}")
                     for r in range(2)]
            h6_sb = [pp.tile([128, KH, C, BL], bf, tag=f"h6{r}", name=f"h6{r}")
                     for r in range(2)]
            dmc_sb = [pp.tile([1, 512], f32, tag=f"dm{r}", name=f"dmc{r}")
                      for r in range(2)]
            dec_sb = [pp.tile([1, 512], f32, tag=f"dec{r}", name=f"dec{r}")
                      for r in range(2)]

            nc.sync.dma_start(u5_sb[:], u5_d.ap().rearrange("k p g -> p k g"))
            nc.sync.dma_start(u6_sb[:], u6_d.ap().rearrange("k p g -> p k g"))
            nc.sync.dma_start(w6_sb[:], w6_d.ap().rearrange("k p g -> p k g"))
            nc.sync.dma_start(w5a_sb[:], w5a_d.ap()[:])
            nc.sync.dma_start(b5_sb[:], b5_d.ap()[:])
            nc.sync.dma_start(b6_sb[:], b6_d.ap()[:])
            nc.sync.dma_start(wd_sb[:], wd_d.ap()[:])
            nc.vector.memset(hz[:], 0.0)
            if has_brh:
                brh5_sb = pp.tile([128, KH], f32, tag="brh5")
                brh6_sb = pp.tile([128, KH], f32, tag="brh6")
                nc.sync.dma_start(brh5_sb[:], br5_d.ap()[:])
                nc.sync.dma_start(brh6_sb[:], br6_d.ap()[:])

            psb = stack.enter_context(
                tc.tile_pool(name="psbulk", bufs=2, space="PSUM"))
            psd = stack.enter_context(
                tc.tile_pool(name="psdense", bufs=1, space="PSUM"))
            prec5 = stack.enter_context(
                tc.tile_pool(name="prec5", bufs=2, space="PSUM"))
            prec6 = stack.enter_context(
                tc.tile_pool(name="prec6", bufs=2, space="PSUM"))
            wk = stack.enter_context(tc.tile_pool(name="wk", bufs=3))

            def bulk5(ring, colbase):
                """xW5 chunk: xaug @ W5aug + b5 -> xw5_sb[ring] (K=1 chunk)."""
                nc.sync.dma_start(
                    xa_sb[ring][:], xt_d.ap()[:, ds(colbase, 512)])
                for cc in range(NCH):
                    ps = psb.tile([128, 512], f32, tag="psb")
                    nc.tensor.matmul(
                        ps[:], w5a_sb[:, cc * 128:(cc + 1) * 128],
                        xa_sb[ring][:], start=True, stop=True,
                    )
                    nc.scalar.activation(
                        xw5_sb[ring][:, :, cc, :],
                        ps[:].rearrange("p (t b) -> p t b", b=BL),
                        AF.Identity, bias=b5_sb[:, cc:cc + 1],
                    )

            def bulk6(ring):
                """xW6 chunk: g5(chunk) @ W6 + b6 -> xw6_sb[ring]."""
                for cc in range(NCH):
                    ps = psb.tile([128, 512], f32, tag="psb")
                    for k in range(KH):
                        nc.tensor.matmul(
                            ps[:], w6_sb[:, k, cc * 128:(cc + 1) * 128],
                            h5_sb[ring][:, k, :, :]
                            .rearrange("p t b -> p (t b)"),
                            start=(k == 0), stop=(k == KH - 1),
                        )
                    nc.scalar.activation(
                        xw6_sb[ring][:, :, cc, :],
                        ps[:].rearrange("p (t b) -> p t b", b=BL),
                        AF.Identity, bias=b6_sb[:, cc:cc + 1],
                    )

            def dense(ring, colbase):
                """dec chunk: tanh(g6 @ Wd + bd) * dec_mask -> out_d."""
                nc.sync.dma_start(
                    dmc_sb[ring][:], dm_d.ap()[:, ds(colbase, 512)])
                ps = psd.tile([1, 512], f32, tag="psd")
                for k in range(KH):
                    nc.tensor.matmul(
                        ps[:], wd_sb[:, k:k + 1],
                        h6_sb[ring][:, k, :, :].rearrange("p t b -> p (t b)"),
                        start=(k == 0), stop=(k == KH - 1),
                    )
                nc.scalar.activation(
                    dec_sb[ring][:], ps[:], AF.Tanh, bias=float(bd_val))
                nc.gpsimd.tensor_tensor(
                    dec_sb[ring][:], dec_sb[ring][:], dmc_sb[ring][:],
                    op=OP.mult)
                nc.sync.dma_start(
                    out_d.ap()[:, ds(colbase, 512)], dec_sb[ring][:])

            def mm_step(lay, ring, uu, hprev):
                """Emit the 48 recurrent matmuls of one step (PE only)."""
                u_sb = u5_sb if lay == 5 else u6_sb
                prec = prec5 if lay == 5 else prec6
                ps_rec = prec.tile([128, 3, KH, BL], f32, tag="psrec")
                ps_zr = ps_rec[:, 0:2, :, :]
                ps_ih = ps_rec[:, 2, :, :]
                first = True
                for g in range(2):          # z, r
                    for cg in range(KH):
                        for k in range(KH):
                            nc.tensor.matmul(
                                ps_zr[:, g, cg, :],
                                u_sb[:, k, g * 512 + cg * 128:
                                     g * 512 + (cg + 1) * 128],
                                hprev(k),
                                start=first,
                                stop=(g == 1 and cg == KH - 1
                                      and k == KH - 1),
                                skip_group_check=True,
                            )
                            first = False
                for cg in range(KH):
                    for k in range(KH):
                        nc.tensor.matmul(
                            ps_ih[:, cg, :],
                            u_sb[:, k, 1024 + cg * 128:1024 + (cg + 1) * 128],
                            hprev(k),
                            start=(cg == 0 and k == 0),
                            stop=(cg == KH - 1 and k == KH - 1),
                            skip_group_check=True,
                        )
                return ps_zr, ps_ih

            def gates_a(lay, ring, uu, ps_zr):
                """z|r merged add + clip -> gzr bf16 (cols: z then r).

                The add reads PSUM so it must run on DVE; layer 6's clip
                runs on gpsimd so L6's chain never head-of-line-blocks
                L5's post-matmul chain in the DVE queue."""
                xw = xw5_sb[ring] if lay == 5 else xw6_sb[ring]
                eng = nc.vector
                szr = wk.tile([128, 2, KH, BL], bf, tag=f"szr{lay}")
                nc.vector.tensor_tensor(
                    szr[:],
                    xw[:, uu, 0:8, :].rearrange("p (g k) b -> p g k b", g=2),
                    ps_zr[:],
                    op=OP.add,
                )
                gzr = wk.tile([128, 2, KH, BL], bf, tag=f"gzr{lay}")
                eng.tensor_scalar(
                    gzr[:], szr[:], 0.0, 1.0, op0=OP.max, op1=OP.min,
                )
                return gzr

            def gates_b(lay, ring, uu, ps_ih, gzr, hprev_full, brh_sb):
                """q, hpre -> tanh; ug/vg on gpsimd."""
                xw = xw5_sb[ring] if lay == 5 else xw6_sb[ring]
                eng = nc.vector
                sh = nc.gpsimd
                q = wk.tile([128, KH, BL], bf, tag=f"q{lay}")
                if brh_sb is not None:
                    nc.vector.scalar_tensor_tensor(
                        q[:], ps_ih[:], 1.0,
                        brh_sb[:].rearrange("p k -> p k 1").broadcast(2, BL),
                        op0=OP.mult, op1=OP.add,
                    )
                    nc.vector.tensor_tensor(
                        q[:], gzr[:, 1, :, :], q[:], op=OP.mult)
                else:
                    nc.vector.tensor_tensor(
                        q[:], gzr[:, 1, :, :], ps_ih[:], op=OP.mult)
                hpre = wk.tile([128, KH, BL], bf, tag=f"hpre{lay}")
                eng.tensor_tensor(
                    hpre[:], q[:], xw[:, uu, 8:12, :], op=OP.add)
                hh = wk.tile([128, KH, BL], bf, tag=f"hh{lay}")
                nc.scalar.activation(hh[:], hpre[:], AF.Tanh)
                # off-spine blend prep, runs in tanh/q shadow on the
                # opposite engine of this layer's chain
                vg = wk.tile([128, KH, BL], bf, tag=f"vg{lay}")
                sh.tensor_scalar(
                    vg[:], gzr[:, 0, :, :], -1.0, 1.0,
                    op0=OP.mult, op1=OP.add,
                )
                ug = wk.tile([128, KH, BL], bf, tag=f"ug{lay}")
                sh.tensor_tensor(
                    ug[:], gzr[:, 0, :, :], hprev_full, op=OP.mult)
                return hh, ug, vg

            def gates_c(lay, ring, uu, hh, ug, vg):
                """ee = vg*hh ; h' = ug + ee -> hist[uu] (bf16)."""
                hist = h5_sb[ring] if lay == 5 else h6_sb[ring]
                eng = nc.vector
                ee = wk.tile([128, KH, BL], bf, tag=f"ee{lay}")
                eng.tensor_tensor(ee[:], vg[:], hh[:], op=OP.mult)
                eng.tensor_tensor(
                    hist[:, :, uu, :], ug[:], ee[:], op=OP.add)

            def hprev_of(lay, ring, uu, zero):
                hist = h5_sb[ring] if lay == 5 else h6_sb[ring]
                prev = h5_sb[1 - ring] if lay == 5 else h6_sb[1 - ring]
                if uu > 0:
                    return (lambda k: hist[:, k, uu - 1, :],
                            hist[:, :, uu - 1, :])
                if zero:
                    return (lambda k: hz[:, k, :], hz[:])
                return (lambda k: prev[:, k, C - 1, :],
                        prev[:, :, C - 1, :])

            brh5 = brh5_sb if has_brh else None
            brh6 = brh6_sb if has_brh else None

            def rec_pair(r5, r6, z5=False, z6=False, do5=True, do6=True):
                """Interleaved chunk: L5 chunk (ring r5) + L6 chunk (r6)."""
                for uu in range(C):
                    if do5:
                        hp5, hp5f = hprev_of(5, r5, uu, z5)
                        zr5, ih5 = mm_step(5, r5, uu, hp5)
                    if do6:
                        hp6, hp6f = hprev_of(6, r6, uu, z6)
                    if do5:
                        g5 = gates_a(5, r5, uu, zr5)
                    if do6:
                        zr6, ih6 = mm_step(6, r6, uu, hp6)
                    if do5:
                        # bias the scheduler to place L5's whole chain
                        # ahead of L6's ops in the DVE/ACT queues, so it
                        # overlaps L6's matmul stream instead of being
                        # stage-paired after it
                        with tc.high_priority(offset=120):
                            hh5, ug5, vg5 = gates_b(5, r5, uu, ih5, g5,
                                                    hp5f, brh5)
                            gates_c(5, r5, uu, hh5, ug5, vg5)
                    if do6:
                        g6 = gates_a(6, r6, uu, zr6)
                        hh6, ug6, vg6 = gates_b(6, r6, uu, ih6, g6, hp6f,
                                                brh6)
                        gates_c(6, r6, uu, hh6, ug6, vg6)

            # ---------------- prologue ----------------
            bulk5(0, 0)
            bulk5(1, 512)
            rec_pair(0, 0, z5=True, do6=False)          # L5 chunk 0
            bulk6(0)
            bulk5(0, 1024)                              # chunk 2
            rec_pair(1, 0, z6=True)                     # L5 c1 | L6 c0
            bulk6(1)
            bulk5(1, 3 * 512)                           # chunk 3
            dense(0, 0)                                 # dec chunk 0

            # ---------------- steady loop: i covers L5 chunks 2..13 -------
            with tc.For_i(
                2 * 512, 14 * 512, 2 * 512,
                hint_engines=(ET.PE, ET.DVE, ET.Activation, ET.Pool),
            ) as iv:
                # first half: L5 chunk c (even, ring0) | L6 chunk c-1 (ring1)
                rec_pair(0, 1)
                bulk6(0)
                bulk5(0, iv + 2 * 512)
                dense(1, iv - 512)
                # second half: L5 chunk c+1 (ring1) | L6 chunk c (ring0)
                rec_pair(1, 0)
                bulk6(1)
                bulk5(1, iv + 3 * 512)
                dense(0, iv)

            # ---------------- epilogue ----------------
            rec_pair(0, 1)                              # L5 c14 | L6 c13
            bulk6(0)
            dense(1, 13 * 512)
            rec_pair(1, 0)                              # L5 c15 | L6 c14
            bulk6(1)
            dense(0, 14 * 512)
            rec_pair(1, 1, do5=False)                   # L6 c15
            dense(1, 15 * 512)

    nc.compile()
    return nc


def _prep(inputs):
    """Host-side: shard on batch, permute/pad/cast into device layouts."""
    z = np.asarray(inputs["z"], np.float32)
    x2 = np.asarray(inputs["train_input_two"], np.float32)
    masks = np.asarray(inputs["masks"], np.float32)
    dmasks = np.asarray(inputs["dec_masks"], np.float32)
    W5 = np.asarray(inputs["W5"], np.float32)
    U5 = np.asarray(inputs["U5"], np.float32)
    bi5 = np.asarray(inputs["bi5"], np.float32)
    br5 = np.asarray(inputs["br5"], np.float32)
    W6 = np.asarray(inputs["W6"], np.float32)
    U6 = np.asarray(inputs["U6"], np.float32)
    bi6 = np.asarray(inputs["bi6"], np.float32)
    br6 = np.asarray(inputs["br6"], np.float32)
    Wd = np.asarray(inputs["Wd"], np.float32)
    bd = np.asarray(inputs["bd"], np.float32)

    def scale_w(W):  # scale z,r columns by 0.2 (hard-sigmoid prescale)
        Ws = W.copy()
        Ws[:, : 2 * H] *= 0.2
        return Ws

    def pack_w(W, kdim):  # [D,G3] -> [kdim,128,G3] bf16 (zero-padded)
        D = W.shape[0]
        Wp = np.zeros((kdim * 128, G3), np.float32)
        Wp[:D] = W
        return np.ascontiguousarray(Wp.reshape(kdim, 128, G3).astype(bf16))

    def pack_bias(bi, br):  # xw-path bias, [128, NCH] (partition, chunk)
        bt = np.empty(G3, np.float32)
        bt[: 2 * H] = 0.2 * (bi[: 2 * H] + br[: 2 * H]) + 0.5
        bt[2 * H:] = bi[2 * H:]
        return np.ascontiguousarray(bt.reshape(NCH, 128).T)

    W5s = scale_w(W5)
    u5p = pack_w(scale_w(U5), KH)
    w6p = pack_w(scale_w(W6), KH)
    u6p = pack_w(scale_w(U6), KH)
    b5p = pack_bias(bi5, br5)
    b6p = pack_bias(bi6, br6)
    brh5 = np.ascontiguousarray(br5[2 * H:].reshape(KH, 128).T)
    brh6 = np.ascontiguousarray(br6[2 * H:].reshape(KH, 128).T)
    has_brh = bool(np.any(brh5) or np.any(brh6))
    wdp = np.ascontiguousarray(Wd[:, 0].reshape(KH, 128).T.astype(bf16))

    in_maps = []
    for cidx in range(NCORES):
        sl = slice(cidx * BL, (cidx + 1) * BL)
        # augmented input: rows 0:64 = (mask*x2)^T, 64:80 = mask*onehot(b)
        xm = x2[sl] * masks[sl]                       # [BL,T,64]
        XT = np.zeros((128, T, BL), np.float32)
        XT[:F2] = xm.transpose(2, 1, 0)
        mk = masks[sl, :, 0]                          # [BL,T]
        for b in range(BL):
            XT[F2 + b, :, b] = mk[b]
        # augmented W5: rows 0:64 = W5s[x2 rows], 64:80 = z @ W5s[z rows]
        W5a = np.zeros((128, G3), np.float32)
        W5a[:F2] = W5s[LAT:]
        W5a[F2:F2 + BL] = z[sl] @ W5s[:LAT]
        dmc = dmasks[sl, :, 0].T.reshape(NT)          # flat t*BL+b
        in_maps.append({
            "xt_d": np.ascontiguousarray(
                XT.reshape(128, NT).astype(bf16)),
            "w5a_d": np.ascontiguousarray(W5a.astype(bf16)),
            "u5_d": u5p, "w6_d": w6p, "u6_d": u6p,
            "b5_d": b5p, "b6_d": b6p,
            "br5_d": brh5, "br6_d": brh6,
            "wd_d": wdp,
            "dm_d": np.ascontiguousarray(dmc.reshape(1, NT)),
        })
    return in_maps, has_brh, float(bd.reshape(-1)[0])


def kernel(**inputs):
    from concourse.bass_utils import run_bass_kernel_spmd

    in_maps, has_brh, bd_val = _prep(inputs)
    key = (has_brh, bd_val)
    if key not in _CACHE:
        _CACHE[key] = _build(bd_val, has_brh)
    nc = _CACHE[key]
    res = run_bass_kernel_spmd(nc, in_maps, core_ids=list(range(NCORES)))
    out = np.empty((B, T, 1), np.float32)
    for cidx in range(NCORES):
        flat = res.results[cidx]["out_d"].reshape(NT)  # flat = t*BL + b
        out[cidx * BL:(cidx + 1) * BL, :, 0] = flat.reshape(T, BL).T
    return out
